# revision 16
# baseline (speedup 1.0000x reference)
"""Graph multi-head attention (GNN message passing) on 8 Trainium2 NeuronCores.

Strategy (dst-sharded edge parallelism, padded edge-stream, no indirect DMA
in the main path):
  - Host: sort edges by dst, split nodes into 8 contiguous ranges with ~equal
    edge counts. Each core owns all incoming edges of its node range, so the
    per-dst segment softmax is core-local.
  - Edges are packed into fixed-width "virtual rows": one row = (node, up to
    D_PAD incoming edges), rows grouped whole-node into 128-row tiles. The
    host stages a PRE-PADDED raw k||v stream in edge-slot order (losslessly
    reordered input data): slot (t, p, s) at stream row t*1024 + p*8 + s.
    Likewise a row-ordered raw q stream.
  - P1/P2 project the streams sequentially with fp16 matmuls (no gather).
    Phase G then reads each supertile's kv/q data with ONE contiguous DMA
    (2KB per descriptor) -- indirect DMA is only used in the tiny F phase
    with the proven [128, 1]-offset form.
  - Bias folding: the K-projection bias shifts all scores of a segment by a
    per-(dst,head) constant, so it cancels in softmax exactly and is dropped.
    The V-projection bias contributes exactly bv per node (sum alpha = 1), so
    it is folded into the output bias: bo' = bo + Wo @ bv.
  - fp16 value path: scores max out around |s| ~ 22, exp(s/4) <= ~300, well
    inside fp16 range; additive mask is -60000 (kills padded slots in exp).
"""

import os
from contextlib import ExitStack

import numpy as np

N = 100000
E = 1600000
DIM = 64
H = 4
DK = DIM // H
NCORES = 8

D_PAD = 8          # edge slots per virtual row
TC = 8             # row-tiles (128 rows each) per supertile
FCH = 8            # node-tiles per F-phase chunk
ROWW = D_PAD * 128  # stream entries per row-tile partition-row group


def _host_prep(src, dst, key, value, query):
    """Pack edges into per-core fixed-shape staged streams."""
    f16 = np.float16

    src = np.asarray(src).astype(np.int64)
    dst = np.asarray(dst).astype(np.int64)
    order = np.argsort(dst, kind="stable")
    ssrc = src[order].astype(np.int64)
    deg = np.bincount(dst, minlength=N).astype(np.int64)
    cum = np.concatenate([[0], np.cumsum(deg)])  # cum[n] = first sorted-edge of n

    bounds = [0]
    for c in range(1, NCORES):
        t = round(c * E / NCORES)
        n = int(np.searchsorted(cum, t, side="left"))
        n = min(max(n, bounds[-1] + 1), N - (NCORES - c))
        bounds.append(n)
    bounds.append(N)

    cores = []
    for c in range(NCORES):
        n0, n1 = bounds[c], bounds[c + 1]
        nn = n1 - n0
        d = deg[n0:n1]
        r_n = np.maximum(1, -(-d // D_PAD)).astype(np.int64)  # rows per node

        # greedy whole-node tiling into 128-row tiles
        tile_of = np.empty(nn, np.int64)
        colrow_of = np.empty(nn, np.int64)  # starting row-slot within tile
        crow_of = np.empty(nn, np.int64)    # node's column id within tile
        t_id = 0
        rows_in_tile = 0
        nodes_in_tile = 0
        for i in range(nn):
            r = r_n[i]
            if rows_in_tile + r > 128:
                t_id += 1
                rows_in_tile = 0
                nodes_in_tile = 0
            tile_of[i] = t_id
            colrow_of[i] = rows_in_tile
            crow_of[i] = nodes_in_tile
            rows_in_tile += r
            nodes_in_tile += 1
        nt_c = t_id + 1
        cores.append(
            dict(n0=n0, n1=n1, nn=nn, d=d, r_n=r_n, tile_of=tile_of,
                 colrow_of=colrow_of, crow_of=crow_of, nt=nt_c)
        )

    NT = max(c["nt"] for c in cores)
    NT = -(-NT // TC) * TC
    NODES_PAD = -(-max(c["nn"] for c in cores) // 128) * 128
    NF = NODES_PAD // 128

    kvraw = np.concatenate([np.asarray(key), np.asarray(value)], axis=1)  # [N,128]
    kvraw16 = kvraw.astype(f16)
    q16 = np.asarray(query).astype(f16)

    for c in cores:
        nn, d, r_n = c["nn"], c["d"], c["r_n"]
        rows_total = NT * 128

        # expand nodes -> rows
        row_node = np.repeat(np.arange(nn), r_n)                     # local node id
        starts = np.concatenate([[0], np.cumsum(r_n)])[:-1]
        row_k = np.arange(len(row_node)) - np.repeat(starts, r_n)    # k-th row of node
        row_slot = (
            np.repeat(c["tile_of"], r_n) * 128
            + np.repeat(c["colrow_of"], r_n) + row_k
        )  # global row index r = t*128 + p
        row_deg = np.clip(np.repeat(d, r_n) - row_k * D_PAD, 0, D_PAD)
        row_e0 = cum[c["n0"] + row_node] + row_k * D_PAD             # first edge
        j = np.arange(D_PAD)[None, :]
        valid = j < row_deg[:, None]
        eidx = np.minimum(row_e0[:, None] + j, E - 1)

        # stream position of (row r, slot s): (r//128)*1024 + s*128 + (r%128)
        spos = ((row_slot // 128) * (128 * D_PAD)
                + (row_slot % 128))[:, None] + j * 128               # [rows, D_PAD]

        # staged raw kv stream, transposed for P1 lhsT: [128 feat, NT*1024]
        kvrawT = np.zeros((128, rows_total * D_PAD), f16)
        kvrawT[:, spos[valid]] = kvraw16[ssrc[eidx[valid]]].T

        # staged raw q stream, row-ordered, with a ones row for the bias
        # (projection contracts over K=65: row 64 of wq_aug is bq)
        qrawT = np.zeros((DIM + 1, rows_total), f16)
        qrawT[:DIM, row_slot] = q16[c["n0"] + row_node].T
        qrawT[DIM, :] = 1.0

        # additive score mask per (row, slot, head): 0 valid, -60000 pad
        # -32 on valid slots folds a global exp shift of e^-8 (softmax-
        # invariant) to keep fp16 exp values in range; -60000 kills pads
        maskf = np.full((rows_total, D_PAD * H), -60000.0, np.float32)
        maskf[row_slot] = np.repeat(
            np.where(valid, -32.0, -60000.0).astype(np.float32), H, axis=1
        )

        crow = np.zeros(rows_total, np.int32)
        crow[row_slot] = np.repeat(c["crow_of"], r_n).astype(np.int32)

        node_ptr = np.zeros(NODES_PAD, np.int32)
        node_ptr[:nn] = (c["tile_of"] * 128 + c["crow_of"]).astype(np.int32)

        c["kvrawT"] = kvrawT
        c["qrawT"] = qrawT
        c["mask"] = (
            maskf.reshape(NT, 128, D_PAD * H)
            .transpose(1, 0, 2)
            .reshape(128, NT * D_PAD * H)
        ).astype(f16).copy()
        c["crow"] = crow.reshape(NT, 128).T.astype(f16).copy()
        c["node_ptr"] = node_ptr.reshape(NF, 128).T.copy()

    return cores, NT, NODES_PAD, NF


def _build_program(NT, NODES_PAD, NF, ST_A, F_A, ST_A2, F_A2):
    import concourse.bass as bass
    import concourse.tile as tile
    from concourse import bacc, mybir
    from concourse.masks import make_identity

    f32 = mybir.dt.float32
    f16 = mybir.dt.float16
    i32 = mybir.dt.int32

    nc = bacc.Bacc("TRN2", target_bir_lowering=False, debug=False,
                   num_devices=NCORES)

    EROWS = NT * 128 * D_PAD   # kv stream entries
    QROWS = NT * 128           # q stream rows

    # inputs
    kvrawT = nc.dram_tensor("kvrawT", [128, EROWS], f16, kind="ExternalInput").ap()
    qrawT = nc.dram_tensor("qrawT", [DIM + 1, QROWS], f16, kind="ExternalInput").ap()
    wkv = nc.dram_tensor("wkv", [128, 128], f16, kind="ExternalInput").ap()
    wqT = nc.dram_tensor("wqT", [DIM + 1, DIM], f16, kind="ExternalInput").ap()
    woT = nc.dram_tensor("woT", [DIM, DIM], f32, kind="ExternalInput").ap()
    bo = nc.dram_tensor("bo", [128, DIM], f32, kind="ExternalInput").ap()
    maskap = nc.dram_tensor("mask", [128, NT * D_PAD * H], f16, kind="ExternalInput").ap()
    crow = nc.dram_tensor("crow", [128, NT], f16, kind="ExternalInput").ap()
    node_ptr = nc.dram_tensor("node_ptr", [128, NF], i32, kind="ExternalInput").ap()
    out = nc.dram_tensor("out", [NODES_PAD, DIM], f32, kind="ExternalOutput").ap()

    comb = nc.dram_tensor("comb", [NT * 128, 68], f16, kind="Internal")
    # first ST_A supertiles' rows are also written here so early F chunks can
    # gather while the main loop is still running (dep only on comb_a writes)
    comb_a = nc.dram_tensor("comb_a", [max(ST_A, 1) * TC * 128, 68], f16,
                            kind="Internal")
    comb_a2 = nc.dram_tensor("comb_a2", [max(ST_A2, 1) * TC * 128, 68], f16,
                             kind="Internal")

    ST = NT // TC
    SLOT = TC * D_PAD

    with tile.TileContext(nc) as tc, ExitStack() as ctx:
        consts = ctx.enter_context(tc.tile_pool(name="consts", bufs=1))
        ld = ctx.enter_context(tc.tile_pool(name="ld", bufs=2))
        gat = ctx.enter_context(tc.tile_pool(name="gat", bufs=3))
        work = ctx.enter_context(tc.tile_pool(name="work", bufs=3))
        smal = ctx.enter_context(tc.tile_pool(name="smal", bufs=3))
        outp = ctx.enter_context(tc.tile_pool(name="outp", bufs=2))
        ps = ctx.enter_context(tc.tile_pool(name="ps", bufs=2, space="PSUM"))

        # constants
        wkv_sb = consts.tile([128, 128], f16)
        nc.sync.dma_start(wkv_sb[:], wkv[:, :])
        wq_sb = consts.tile([DIM + 1, DIM], f16)
        nc.sync.dma_start(wq_sb[:], wqT[:, :])
        wo_sb = consts.tile([DIM, DIM], f32)
        nc.sync.dma_start(wo_sb[:], woT[:, :])
        bo_sb = consts.tile([128, DIM], f32)
        nc.sync.dma_start(bo_sb[:], bo[:, :])
        ident = consts.tile([128, 128], f32)
        make_identity(nc, ident[:])
        iota_i = consts.tile([128, 128], i32)
        nc.gpsimd.iota(iota_i[:], pattern=[[1, 128]], base=0, channel_multiplier=0)
        iota_f = consts.tile([128, 128], f16)
        nc.vector.tensor_copy(iota_f[:], iota_i[:])

        def ap4(t, dims, extra_off=0):
            a = t[:]
            return bass.AP(a.tensor, a.offset + extra_off, [list(a.ap[0])] + dims)

        tl = {}  # per-supertile live tiles

        def stage_P(st):
            """Load raw streams, project into SBUF (fused P1/P2)."""
            d = {}
            lhs = ld.tile([128, TC * ROWW], f16, tag="kt")
            nc.sync.dma_start(lhs[:], kvrawT[:, st * TC * ROWW : (st + 1) * TC * ROWW])
            qld = ld.tile([DIM + 1, TC * 128], f16, tag="qt")
            nc.sync.dma_start(qld[:], qrawT[:, st * TC * 128 : (st + 1) * TC * 128])
            d["msk"] = smal.tile([128, SLOT * H], f16, tag="msk", name=f"msk{st}")
            nc.sync.dma_start(d["msk"][:],
                              maskap[:, st * SLOT * H : (st + 1) * SLOT * H])
            d["crw"] = smal.tile([128, TC], f16, tag="crw", name=f"crw{st}")
            nc.sync.dma_start(d["crw"][:], crow[:, st * TC : (st + 1) * TC])

            kv_g = gat.tile([128, TC, ROWW], f16, tag="kv_g")
            for t in range(TC):
                for half in range(2):
                    acc = ps.tile([128, 4 * 128], f32, space="PSUM", tag="p1")
                    for s4 in range(4):
                        jj = t * D_PAD + half * 4 + s4
                        nc.tensor.matmul(
                            out=acc[:, s4 * 128 : (s4 + 1) * 128],
                            lhsT=lhs[:, jj * 128 : (jj + 1) * 128],
                            rhs=wkv_sb[:], start=True, stop=True)
                    nc.scalar.activation(
                        out=kv_g[:, t, half * 512 : (half + 1) * 512], in_=acc[:],
                        func=mybir.ActivationFunctionType.Copy)
            d["kv_g"] = kv_g
            q_g = smal.tile([128, TC, DIM], f16, tag="q_g")
            for t0 in range(0, TC, 4):
                acc = ps.tile([128, 4 * DIM], f32, space="PSUM", tag="mm")
                for t in range(t0, t0 + 4):
                    nc.tensor.matmul(
                        out=acc[:, (t - t0) * DIM : (t - t0 + 1) * DIM],
                        lhsT=qld[:, t * 128 : (t + 1) * 128],
                        rhs=wq_sb[:], start=True, stop=True)
                nc.scalar.activation(
                    out=q_g[:, t0 : t0 + 4, :], in_=acc[:],
                    func=mybir.ActivationFunctionType.Copy)
            d["q_g"] = q_g
            tl[st] = d

        def stage_A(st):
            """Scores: prod, per-head reduce, mask add (DVE)."""
            d = tl[st]
            kv_g, q_g = d["kv_g"], d["q_g"]
            prod = work.tile([128, SLOT, DIM], f16, tag="prod")
            for sl in range(D_PAD):
                nc.vector.tensor_tensor(
                    out=ap4(prod, [[D_PAD * DIM, TC], [1, DIM]], extra_off=sl * DIM),
                    in0=ap4(kv_g, [[ROWW, TC], [1, DIM]], extra_off=sl * 128),
                    in1=ap4(q_g, [[DIM, TC], [1, DIM]]),
                    op=mybir.AluOpType.mult,
                )
            d["prod"] = prod
            # per-head 16-dim dot via in-place strided tree adds (fp16 2x;
            # TENSOR_REDUCE only runs at 1x with these APs)
            sco = smal.tile([128, SLOT, H], f16, tag="sco")
            nc.vector.tensor_tensor(
                out=ap4(prod, [[DIM, SLOT], [DK, H], [1, 8]]),
                in0=ap4(prod, [[DIM, SLOT], [DK, H], [1, 8]]),
                in1=ap4(prod, [[DIM, SLOT], [DK, H], [1, 8]], extra_off=8),
                op=mybir.AluOpType.add,
            )
            nc.vector.tensor_tensor(
                out=ap4(prod, [[DIM, SLOT], [DK, H], [1, 4]]),
                in0=ap4(prod, [[DIM, SLOT], [DK, H], [1, 4]]),
                in1=ap4(prod, [[DIM, SLOT], [DK, H], [1, 4]], extra_off=4),
                op=mybir.AluOpType.add,
            )
            nc.vector.tensor_tensor(
                out=ap4(prod, [[DIM, SLOT], [DK, H], [1, 2]]),
                in0=ap4(prod, [[DIM, SLOT], [DK, H], [1, 2]]),
                in1=ap4(prod, [[DIM, SLOT], [DK, H], [1, 2]], extra_off=2),
                op=mybir.AluOpType.add,
            )
            nc.vector.tensor_tensor(
                out=ap4(sco, [[H, SLOT], [1, H]]),
                in0=ap4(prod, [[DIM, SLOT], [16, H]]),
                in1=ap4(prod, [[DIM, SLOT], [16, H]], extra_off=1),
                op=mybir.AluOpType.add,
            )
            nc.vector.tensor_tensor(
                out=ap4(sco, [[1, SLOT * H]]),
                in0=ap4(sco, [[1, SLOT * H]]),
                in1=d["msk"][:], op=mybir.AluOpType.add,
            )
            d["sco"] = sco

        def stage_B(st):
            """exp (ACT)."""
            d = tl[st]
            ex = smal.tile([128, SLOT, H], f16, tag="ex")
            nc.scalar.activation(
                out=ex[:], in_=d["sco"][:],
                func=mybir.ActivationFunctionType.Exp,
                scale=1.0 / np.sqrt(DK),
            )
            d["ex"] = ex

        def stage_C(st):
            """Weighted V, slot trees, one-hot combine, comb write."""
            d = tl.pop(st)
            kv_g, ex, wv = d["kv_g"], d["ex"], d["prod"]
            # wv[p,t,s,f] = v'[p,t,s,f] * ex[p,t,s,h(f)]  (4-dim ex broadcast)
            for sl in range(D_PAD):
                nc.vector.tensor_tensor(
                    out=ap4(wv, [[D_PAD * DIM, TC], [1, DIM]], extra_off=sl * DIM),
                    in0=ap4(kv_g, [[ROWW, TC], [1, DIM]], extra_off=sl * 128 + DIM),
                    in1=ap4(ex, [[D_PAD * H, TC], [1, H], [0, DK]], extra_off=sl * H),
                    op=mybir.AluOpType.mult,
                )
            ad = smal.tile([128, TC, 68], f16, tag="ad")
            nc.vector.tensor_tensor(
                out=ap4(wv, [[D_PAD * DIM, TC], [DIM, 4], [1, DIM]]),
                in0=ap4(wv, [[D_PAD * DIM, TC], [DIM, 4], [1, DIM]]),
                in1=ap4(wv, [[D_PAD * DIM, TC], [DIM, 4], [1, DIM]],
                        extra_off=4 * DIM),
                op=mybir.AluOpType.add,
            )
            nc.vector.tensor_tensor(
                out=ap4(wv, [[D_PAD * DIM, TC], [DIM, 2], [1, DIM]]),
                in0=ap4(wv, [[D_PAD * DIM, TC], [DIM, 2], [1, DIM]]),
                in1=ap4(wv, [[D_PAD * DIM, TC], [DIM, 2], [1, DIM]],
                        extra_off=2 * DIM),
                op=mybir.AluOpType.add,
            )
            nc.vector.tensor_tensor(
                out=ap4(ad, [[68, TC], [1, DIM]]),
                in0=ap4(wv, [[D_PAD * DIM, TC], [1, DIM]]),
                in1=ap4(wv, [[D_PAD * DIM, TC], [1, DIM]], extra_off=DIM),
                op=mybir.AluOpType.add,
            )
            nc.vector.tensor_tensor(
                out=ap4(ex, [[D_PAD * H, TC], [H, 4], [1, H]]),
                in0=ap4(ex, [[D_PAD * H, TC], [H, 4], [1, H]]),
                in1=ap4(ex, [[D_PAD * H, TC], [H, 4], [1, H]], extra_off=4 * H),
                op=mybir.AluOpType.add,
            )
            nc.vector.tensor_tensor(
                out=ap4(ex, [[D_PAD * H, TC], [H, 2], [1, H]]),
                in0=ap4(ex, [[D_PAD * H, TC], [H, 2], [1, H]]),
                in1=ap4(ex, [[D_PAD * H, TC], [H, 2], [1, H]], extra_off=2 * H),
                op=mybir.AluOpType.add,
            )
            nc.vector.tensor_tensor(
                out=ap4(ad, [[68, TC], [1, H]], extra_off=DIM),
                in0=ap4(ex, [[D_PAD * H, TC], [1, H]]),
                in1=ap4(ex, [[D_PAD * H, TC], [1, H]], extra_off=H),
                op=mybir.AluOpType.add,
            )
            oh_all = work.tile([128, TC, 128], f16, tag="oh")
            nc.vector.tensor_tensor(
                out=oh_all[:, :, :],
                in0=ap4(iota_f, [[0, TC], [1, 128]]),
                in1=ap4(d["crw"], [[1, TC], [0, 128]]),
                op=mybir.AluOpType.is_equal,
            )
            csb_all = outp.tile([128, TC, 68], f16, tag="csb")
            for t in range(TC):
                cps = ps.tile([128, 68], f32, space="PSUM", tag="x")
                nc.tensor.matmul(out=cps[:], lhsT=oh_all[:, t, :],
                                 rhs=ad[:, t, :], start=True, stop=True)
                nc.scalar.activation(
                    out=csb_all[:, t, :], in_=cps[:],
                    func=mybir.ActivationFunctionType.Copy,
                )
            nc.sync.dma_start(
                bass.AP(comb.ap()[:, :].tensor, st * TC * 128 * 68,
                        [[68, 128], [128 * 68, TC], [1, 68]]),
                csb_all[:, :, :],
            )
            if st < ST_A:
                nc.sync.dma_start(
                    bass.AP(comb_a.ap()[:, :].tensor, st * TC * 128 * 68,
                            [[68, 128], [128 * 68, TC], [1, 68]]),
                    csb_all[:, :, :],
                )
            if st < ST_A2:
                nc.sync.dma_start(
                    bass.AP(comb_a2.ap()[:, :].tensor, st * TC * 128 * 68,
                            [[68, 128], [128 * 68, TC], [1, 68]]),
                    csb_all[:, :, :],
                )

        nptr = consts.tile([128, NF], i32)
        nc.sync.dma_start(nptr[:], node_ptr[:, :])

        def f_chunk(fc, w, src_t):
            cgc = smal.tile([128, FCH, 68], f16, tag="cgc",
                            name=f"cgc{fc}")
            for f in range(w):
                nc.gpsimd.indirect_dma_start(
                    out=cgc[:, f, :], out_offset=None, in_=src_t.ap()[:, :],
                    in_offset=bass.IndirectOffsetOnAxis(
                        ap=nptr[:, fc + f : fc + f + 1], axis=0),
                )
            dn = smal.tile([128, FCH, H], f32, tag="dn", name=f"dn{fc}")
            nc.vector.tensor_scalar(
                out=dn[:, 0:w, :],
                in0=ap4(cgc, [[68, w], [1, H]], extra_off=DIM),
                scalar1=1e-30, scalar2=None,
                op0=mybir.AluOpType.max,
            )
            rd = smal.tile([128, FCH, H], f32, tag="rd", name=f"rd{fc}")
            nc.vector.reciprocal(rd[:, 0:w, :], dn[:, 0:w, :])
            nrm = outp.tile([128, FCH, DIM], f32, tag="nrm", name=f"nrm{fc}")
            nc.vector.tensor_tensor(
                out=nrm[:, 0:w, :],
                in0=ap4(cgc, [[68, w], [1, DIM]]),
                in1=ap4(rd, [[H, w], [1, H], [0, DK]]),
                op=mybir.AluOpType.mult,
            )
            osb = outp.tile([128, FCH, DIM], f32, tag="osb", name=f"osb{fc}")
            for f in range(w):
                tps = ps.tile([DIM, 128], f32, space="PSUM", tag="x",
                              name=f"tps{fc}_{f}")
                nc.tensor.transpose(out=tps[:], in_=nrm[:, f, :],
                                    identity=ident[:])
                nrmT = outp.tile([DIM, 128], f32, tag="nrmT",
                                 name=f"nrmT{fc}_{f}")
                nc.scalar.activation(
                    out=nrmT[:], in_=tps[:],
                    func=mybir.ActivationFunctionType.Copy,
                )
                ops_ = ps.tile([128, DIM], f32, space="PSUM", tag="mm",
                               name=f"ops{fc}_{f}")
                nc.tensor.matmul(out=ops_[:], lhsT=nrmT[:], rhs=wo_sb[:],
                                 start=True, stop=True)
                nc.vector.tensor_tensor(
                    out=osb[:, f, :], in0=ops_[:], in1=bo_sb[:],
                    op=mybir.AluOpType.add,
                )
            nc.sync.dma_start(
                bass.AP(out.tensor, fc * 128 * DIM,
                        [[DIM, 128], [128 * DIM, w], [1, DIM]]),
                osb[:, 0:w, :],
            )

        # software-pipelined main loop (2-supertile skew)
        WARM = min(2, ST)
        for st in range(WARM):
            stage_P(st)
            stage_A(st)
            stage_B(st)
        a_chunks = list(range(0, F_A, FCH))
        a2_chunks = list(range(F_A, F_A2, FCH))
        for st in range(ST):
            stage_C(st)
            if st >= ST_A and st % 2 == 1 and a_chunks:
                fc = a_chunks.pop(0)
                f_chunk(fc, min(FCH, NF - fc), comb_a)
            if st >= ST_A2 and st % 2 == 0 and a2_chunks:
                fc = a2_chunks.pop(0)
                f_chunk(fc, min(FCH, NF - fc), comb_a2)
            nx = st + WARM
            if nx < ST:
                stage_P(nx)
                stage_A(nx)
                stage_B(nx)
        for fc in a_chunks:
            f_chunk(fc, min(FCH, NF - fc), comb_a)
        for fc in a2_chunks:
            f_chunk(fc, min(FCH, NF - fc), comb_a2)

        # ---- Phase F chunks come from f_chunk(); A-chunks (early node
        # tiles, rows all inside comb_a) were emitted inside the main loop ----
        for fc in range(F_A2, NF, FCH):
            f_chunk(fc, min(FCH, NF - fc), comb)

    nc.compile()
    return nc


def kernel(**inputs):
    from concourse.bass_utils import run_bass_kernel_spmd

    f16 = np.float16

    query = np.asarray(inputs["query"], np.float32)
    key = np.asarray(inputs["key"], np.float32)
    value = np.asarray(inputs["value"], np.float32)
    src = np.asarray(inputs["src"])
    dst = np.asarray(inputs["dst"])
    Wq = np.asarray(inputs["Wq"], np.float32)
    bq = np.asarray(inputs["bq"], np.float32)
    Wk = np.asarray(inputs["Wk"], np.float32)
    bk = np.asarray(inputs["bk"], np.float32)  # noqa: F841  (cancels in softmax)
    Wv = np.asarray(inputs["Wv"], np.float32)
    bv = np.asarray(inputs["bv"], np.float32)
    Wo = np.asarray(inputs["Wo"], np.float32)
    bo = np.asarray(inputs["bo"], np.float32)

    cores, NT, NODES_PAD, NF = _host_prep(src, dst, key, value, query)
    ST = NT // TC

    def safe_split(st_a):
        # largest F bound (multiple of FCH) with every core's rows for those
        # nodes inside tiles < st_a*TC
        if st_a < 1:
            return 0
        lim = st_a * TC
        fa = NF
        for c in cores:
            tf = c["tile_of"]
            ok = np.nonzero(tf >= lim)[0]
            first_bad = int(ok[0]) if len(ok) else c["nn"]
            fa = min(fa, first_bad // 128)
        return max(0, (fa // FCH) * FCH)

    ST_A = ST // 2
    ST_A2 = min(ST - 7, max(ST_A + 1, (3 * ST) // 4)) if ST >= 8 else ST_A
    F_A = safe_split(ST_A)
    F_A2 = max(F_A, safe_split(ST_A2))
    nc = _build_program(NT, NODES_PAD, NF, ST_A, F_A, ST_A2, F_A2)

    wkv = np.zeros((128, 128), f16)
    wkv[0:DIM, 0:DIM] = Wk.T.astype(f16)
    wkv[DIM:128, DIM:128] = Wv.T.astype(f16)
    # bias folding: bk cancels in segment softmax; bv contributes Wo@bv to out
    boP = bo + Wo @ bv

    wq_aug = np.concatenate([Wq.T, bq[None, :]], axis=0).astype(f16)  # [65, 64]

    in_maps = []
    for c in cores:
        in_maps.append(
            dict(
                kvrawT=c["kvrawT"], qrawT=c["qrawT"], wkv=wkv,
                wqT=wq_aug.copy(),
                woT=Wo.T.copy(),
                bo=np.broadcast_to(boP, (128, DIM)).astype(np.float32).copy(),
                mask=c["mask"],
                crow=c["crow"], node_ptr=c["node_ptr"],
            )
        )

    trace = bool(int(os.environ.get("KERNEL_TRACE", "0")))
    res = run_bass_kernel_spmd(
        nc, in_maps, core_ids=list(range(NCORES)), trace=trace,
        tmpdir=os.environ.get("KERNEL_TRACE_DIR") or None,
    )
    kernel.last_results = res

    out = np.empty((N, DIM), np.float32)
    for c, r in zip(cores, res.results):
        out[c["n0"] : c["n1"]] = r["out"][: c["nn"]]
    # deg-0 nodes have sum(alpha) = 0, so the folded Wo@bv term must be removed
    deg = np.bincount(np.asarray(dst).astype(np.int64), minlength=N)
    z = deg == 0
    if z.any():
        out[z] -= Wo @ bv
    return out


# revision 17
# speedup vs baseline: 1.0051x; 1.0051x over previous
"""Graph multi-head attention (GNN message passing) on 8 Trainium2 NeuronCores.

Strategy (dst-sharded edge parallelism, padded edge-stream, no indirect DMA
in the main path):
  - Host: sort edges by dst, split nodes into 8 contiguous ranges with ~equal
    edge counts. Each core owns all incoming edges of its node range, so the
    per-dst segment softmax is core-local.
  - Edges are packed into fixed-width "virtual rows": one row = (node, up to
    D_PAD incoming edges), rows grouped whole-node into 128-row tiles. The
    host stages a PRE-PADDED raw k||v stream in edge-slot order (losslessly
    reordered input data): slot (t, p, s) at stream row t*1024 + p*8 + s.
    Likewise a row-ordered raw q stream.
  - P1/P2 project the streams sequentially with fp16 matmuls (no gather).
    Phase G then reads each supertile's kv/q data with ONE contiguous DMA
    (2KB per descriptor) -- indirect DMA is only used in the tiny F phase
    with the proven [128, 1]-offset form.
  - Bias folding: the K-projection bias shifts all scores of a segment by a
    per-(dst,head) constant, so it cancels in softmax exactly and is dropped.
    The V-projection bias contributes exactly bv per node (sum alpha = 1), so
    it is folded into the output bias: bo' = bo + Wo @ bv.
  - fp16 value path: scores max out around |s| ~ 22, exp(s/4) <= ~300, well
    inside fp16 range; additive mask is -60000 (kills padded slots in exp).
"""

import os
from contextlib import ExitStack

import numpy as np

N = 100000
E = 1600000
DIM = 64
H = 4
DK = DIM // H
NCORES = 8

D_PAD = 8          # edge slots per virtual row
TC = 8             # row-tiles (128 rows each) per supertile
FCH = 8            # node-tiles per F-phase chunk
ROWW = D_PAD * 128  # stream entries per row-tile partition-row group


def _host_prep(src, dst, key, value, query):
    """Pack edges into per-core fixed-shape staged streams."""
    f16 = np.float16

    src = np.asarray(src).astype(np.int64)
    dst = np.asarray(dst).astype(np.int64)
    order = np.argsort(dst, kind="stable")
    ssrc = src[order].astype(np.int64)
    deg = np.bincount(dst, minlength=N).astype(np.int64)
    cum = np.concatenate([[0], np.cumsum(deg)])  # cum[n] = first sorted-edge of n

    bounds = [0]
    for c in range(1, NCORES):
        t = round(c * E / NCORES)
        n = int(np.searchsorted(cum, t, side="left"))
        n = min(max(n, bounds[-1] + 1), N - (NCORES - c))
        bounds.append(n)
    bounds.append(N)

    cores = []
    for c in range(NCORES):
        n0, n1 = bounds[c], bounds[c + 1]
        nn = n1 - n0
        d = deg[n0:n1]
        r_n = np.maximum(1, -(-d // D_PAD)).astype(np.int64)  # rows per node

        # greedy whole-node tiling into 128-row tiles
        tile_of = np.empty(nn, np.int64)
        colrow_of = np.empty(nn, np.int64)  # starting row-slot within tile
        crow_of = np.empty(nn, np.int64)    # node's column id within tile
        t_id = 0
        rows_in_tile = 0
        nodes_in_tile = 0
        for i in range(nn):
            r = r_n[i]
            if rows_in_tile + r > 128:
                t_id += 1
                rows_in_tile = 0
                nodes_in_tile = 0
            tile_of[i] = t_id
            colrow_of[i] = rows_in_tile
            crow_of[i] = nodes_in_tile
            rows_in_tile += r
            nodes_in_tile += 1
        nt_c = t_id + 1
        cores.append(
            dict(n0=n0, n1=n1, nn=nn, d=d, r_n=r_n, tile_of=tile_of,
                 colrow_of=colrow_of, crow_of=crow_of, nt=nt_c)
        )

    NT = max(c["nt"] for c in cores)
    NT = -(-NT // TC) * TC
    NODES_PAD = -(-max(c["nn"] for c in cores) // 128) * 128
    NF = NODES_PAD // 128

    kvraw = np.concatenate([np.asarray(key), np.asarray(value)], axis=1)  # [N,128]
    kvraw16 = kvraw.astype(f16)
    q16 = np.asarray(query).astype(f16)

    for c in cores:
        nn, d, r_n = c["nn"], c["d"], c["r_n"]
        rows_total = NT * 128

        # expand nodes -> rows
        row_node = np.repeat(np.arange(nn), r_n)                     # local node id
        starts = np.concatenate([[0], np.cumsum(r_n)])[:-1]
        row_k = np.arange(len(row_node)) - np.repeat(starts, r_n)    # k-th row of node
        row_slot = (
            np.repeat(c["tile_of"], r_n) * 128
            + np.repeat(c["colrow_of"], r_n) + row_k
        )  # global row index r = t*128 + p
        row_deg = np.clip(np.repeat(d, r_n) - row_k * D_PAD, 0, D_PAD)
        row_e0 = cum[c["n0"] + row_node] + row_k * D_PAD             # first edge
        j = np.arange(D_PAD)[None, :]
        valid = j < row_deg[:, None]
        eidx = np.minimum(row_e0[:, None] + j, E - 1)

        # stream position of (row r, slot s): (r//128)*1024 + s*128 + (r%128)
        spos = ((row_slot // 128) * (128 * D_PAD)
                + (row_slot % 128))[:, None] + j * 128               # [rows, D_PAD]

        # staged raw kv stream, transposed for P1 lhsT: [128 feat, NT*1024]
        kvrawT = np.zeros((128, rows_total * D_PAD), f16)
        kvrawT[:, spos[valid]] = kvraw16[ssrc[eidx[valid]]].T

        # staged raw q stream, row-ordered, with a ones row for the bias
        # (projection contracts over K=65: row 64 of wq_aug is bq)
        qrawT = np.zeros((DIM + 1, rows_total), f16)
        qrawT[:DIM, row_slot] = q16[c["n0"] + row_node].T
        qrawT[DIM, :] = 1.0

        # additive score mask per (row, slot, head): 0 valid, -60000 pad
        # -32 on valid slots folds a global exp shift of e^-8 (softmax-
        # invariant) to keep fp16 exp values in range; -60000 kills pads
        maskf = np.full((rows_total, D_PAD * H), -60000.0, np.float32)
        maskf[row_slot] = np.repeat(
            np.where(valid, -32.0, -60000.0).astype(np.float32), H, axis=1
        )

        crow = np.zeros(rows_total, np.int32)
        crow[row_slot] = np.repeat(c["crow_of"], r_n).astype(np.int32)

        node_ptr = np.zeros(NODES_PAD, np.int32)
        node_ptr[:nn] = (c["tile_of"] * 128 + c["crow_of"]).astype(np.int32)

        c["kvrawT"] = kvrawT
        c["qrawT"] = qrawT
        c["mask"] = (
            maskf.reshape(NT, 128, D_PAD * H)
            .transpose(1, 0, 2)
            .reshape(128, NT * D_PAD * H)
        ).astype(f16).copy()
        c["crow"] = crow.reshape(NT, 128).T.astype(f16).copy()
        c["node_ptr"] = node_ptr.reshape(NF, 128).T.copy()

    return cores, NT, NODES_PAD, NF


def _build_program(NT, NODES_PAD, NF, ST_A, F_A):
    import concourse.bass as bass
    import concourse.tile as tile
    from concourse import bacc, mybir
    from concourse.masks import make_identity

    f32 = mybir.dt.float32
    f16 = mybir.dt.float16
    i32 = mybir.dt.int32

    nc = bacc.Bacc("TRN2", target_bir_lowering=False, debug=False,
                   num_devices=NCORES)

    EROWS = NT * 128 * D_PAD   # kv stream entries
    QROWS = NT * 128           # q stream rows

    # inputs
    kvrawT = nc.dram_tensor("kvrawT", [128, EROWS], f16, kind="ExternalInput").ap()
    qrawT = nc.dram_tensor("qrawT", [DIM + 1, QROWS], f16, kind="ExternalInput").ap()
    wkv = nc.dram_tensor("wkv", [128, 128], f16, kind="ExternalInput").ap()
    wqT = nc.dram_tensor("wqT", [DIM + 1, DIM], f16, kind="ExternalInput").ap()
    woT = nc.dram_tensor("woT", [DIM, DIM], f32, kind="ExternalInput").ap()
    bo = nc.dram_tensor("bo", [128, DIM], f32, kind="ExternalInput").ap()
    maskap = nc.dram_tensor("mask", [128, NT * D_PAD * H], f16, kind="ExternalInput").ap()
    crow = nc.dram_tensor("crow", [128, NT], f16, kind="ExternalInput").ap()
    node_ptr = nc.dram_tensor("node_ptr", [128, NF], i32, kind="ExternalInput").ap()
    out = nc.dram_tensor("out", [NODES_PAD, DIM], f32, kind="ExternalOutput").ap()

    comb = nc.dram_tensor("comb", [NT * 128, 68], f16, kind="Internal")
    # first ST_A supertiles' rows are also written here so early F chunks can
    # gather while the main loop is still running (dep only on comb_a writes)
    comb_a = nc.dram_tensor("comb_a", [max(ST_A, 1) * TC * 128, 68], f16,
                            kind="Internal")

    ST = NT // TC
    SLOT = TC * D_PAD

    with tile.TileContext(nc) as tc, ExitStack() as ctx:
        consts = ctx.enter_context(tc.tile_pool(name="consts", bufs=1))
        ld = ctx.enter_context(tc.tile_pool(name="ld", bufs=2))
        gat = ctx.enter_context(tc.tile_pool(name="gat", bufs=3))
        work = ctx.enter_context(tc.tile_pool(name="work", bufs=3))
        smal = ctx.enter_context(tc.tile_pool(name="smal", bufs=3))
        outp = ctx.enter_context(tc.tile_pool(name="outp", bufs=2))
        ps = ctx.enter_context(tc.tile_pool(name="ps", bufs=2, space="PSUM"))

        # constants
        wkv_sb = consts.tile([128, 128], f16)
        nc.sync.dma_start(wkv_sb[:], wkv[:, :])
        wq_sb = consts.tile([DIM + 1, DIM], f16)
        nc.sync.dma_start(wq_sb[:], wqT[:, :])
        wo_sb = consts.tile([DIM, DIM], f32)
        nc.sync.dma_start(wo_sb[:], woT[:, :])
        bo_sb = consts.tile([128, DIM], f32)
        nc.sync.dma_start(bo_sb[:], bo[:, :])
        ident = consts.tile([128, 128], f32)
        make_identity(nc, ident[:])
        iota_i = consts.tile([128, 128], i32)
        nc.gpsimd.iota(iota_i[:], pattern=[[1, 128]], base=0, channel_multiplier=0)
        iota_f = consts.tile([128, 128], f16)
        nc.vector.tensor_copy(iota_f[:], iota_i[:])

        def ap4(t, dims, extra_off=0):
            a = t[:]
            return bass.AP(a.tensor, a.offset + extra_off, [list(a.ap[0])] + dims)

        tl = {}  # per-supertile live tiles

        def stage_P(st):
            """Load raw streams, project into SBUF (fused P1/P2)."""
            d = {}
            lhs = ld.tile([128, TC * ROWW], f16, tag="kt")
            nc.sync.dma_start(lhs[:], kvrawT[:, st * TC * ROWW : (st + 1) * TC * ROWW])
            qld = ld.tile([DIM + 1, TC * 128], f16, tag="qt")
            nc.sync.dma_start(qld[:], qrawT[:, st * TC * 128 : (st + 1) * TC * 128])
            d["msk"] = smal.tile([128, SLOT * H], f16, tag="msk", name=f"msk{st}")
            nc.sync.dma_start(d["msk"][:],
                              maskap[:, st * SLOT * H : (st + 1) * SLOT * H])
            d["crw"] = smal.tile([128, TC], f16, tag="crw", name=f"crw{st}")
            nc.sync.dma_start(d["crw"][:], crow[:, st * TC : (st + 1) * TC])

            kv_g = gat.tile([128, TC, ROWW], f16, tag="kv_g")
            for t in range(TC):
                for half in range(2):
                    acc = ps.tile([128, 4 * 128], f32, space="PSUM", tag="p1")
                    for s4 in range(4):
                        jj = t * D_PAD + half * 4 + s4
                        nc.tensor.matmul(
                            out=acc[:, s4 * 128 : (s4 + 1) * 128],
                            lhsT=lhs[:, jj * 128 : (jj + 1) * 128],
                            rhs=wkv_sb[:], start=True, stop=True)
                    nc.scalar.activation(
                        out=kv_g[:, t, half * 512 : (half + 1) * 512], in_=acc[:],
                        func=mybir.ActivationFunctionType.Copy)
            d["kv_g"] = kv_g
            q_g = smal.tile([128, TC, DIM], f16, tag="q_g")
            for t0 in range(0, TC, 4):
                acc = ps.tile([128, 4 * DIM], f32, space="PSUM", tag="mm")
                for t in range(t0, t0 + 4):
                    nc.tensor.matmul(
                        out=acc[:, (t - t0) * DIM : (t - t0 + 1) * DIM],
                        lhsT=qld[:, t * 128 : (t + 1) * 128],
                        rhs=wq_sb[:], start=True, stop=True)
                nc.scalar.activation(
                    out=q_g[:, t0 : t0 + 4, :], in_=acc[:],
                    func=mybir.ActivationFunctionType.Copy)
            d["q_g"] = q_g
            tl[st] = d

        def stage_A(st):
            """Scores: prod, per-head reduce, mask add (DVE)."""
            d = tl[st]
            kv_g, q_g = d["kv_g"], d["q_g"]
            prod = work.tile([128, SLOT, DIM], f16, tag="prod")
            for sl in range(D_PAD):
                nc.vector.tensor_tensor(
                    out=ap4(prod, [[D_PAD * DIM, TC], [1, DIM]], extra_off=sl * DIM),
                    in0=ap4(kv_g, [[ROWW, TC], [1, DIM]], extra_off=sl * 128),
                    in1=ap4(q_g, [[DIM, TC], [1, DIM]]),
                    op=mybir.AluOpType.mult,
                )
            d["prod"] = prod
            # per-head 16-dim dot via in-place strided tree adds (fp16 2x;
            # TENSOR_REDUCE only runs at 1x with these APs)
            sco = smal.tile([128, SLOT, H], f16, tag="sco")
            nc.vector.tensor_tensor(
                out=ap4(prod, [[DIM, SLOT], [DK, H], [1, 8]]),
                in0=ap4(prod, [[DIM, SLOT], [DK, H], [1, 8]]),
                in1=ap4(prod, [[DIM, SLOT], [DK, H], [1, 8]], extra_off=8),
                op=mybir.AluOpType.add,
            )
            nc.vector.tensor_tensor(
                out=ap4(prod, [[DIM, SLOT], [DK, H], [1, 4]]),
                in0=ap4(prod, [[DIM, SLOT], [DK, H], [1, 4]]),
                in1=ap4(prod, [[DIM, SLOT], [DK, H], [1, 4]], extra_off=4),
                op=mybir.AluOpType.add,
            )
            nc.vector.tensor_tensor(
                out=ap4(prod, [[DIM, SLOT], [DK, H], [1, 2]]),
                in0=ap4(prod, [[DIM, SLOT], [DK, H], [1, 2]]),
                in1=ap4(prod, [[DIM, SLOT], [DK, H], [1, 2]], extra_off=2),
                op=mybir.AluOpType.add,
            )
            nc.vector.tensor_tensor(
                out=ap4(sco, [[H, SLOT], [1, H]]),
                in0=ap4(prod, [[DIM, SLOT], [16, H]]),
                in1=ap4(prod, [[DIM, SLOT], [16, H]], extra_off=1),
                op=mybir.AluOpType.add,
            )
            nc.vector.tensor_tensor(
                out=ap4(sco, [[1, SLOT * H]]),
                in0=ap4(sco, [[1, SLOT * H]]),
                in1=d["msk"][:], op=mybir.AluOpType.add,
            )
            d["sco"] = sco

        def stage_B(st):
            """exp (ACT)."""
            d = tl[st]
            ex = smal.tile([128, SLOT, H], f16, tag="ex")
            nc.scalar.activation(
                out=ex[:], in_=d["sco"][:],
                func=mybir.ActivationFunctionType.Exp,
                scale=1.0 / np.sqrt(DK),
            )
            d["ex"] = ex

        def stage_C(st):
            """Weighted V, slot trees, one-hot combine, comb write."""
            d = tl.pop(st)
            kv_g, ex, wv = d["kv_g"], d["ex"], d["prod"]
            # wv[p,t,s,f] = v'[p,t,s,f] * ex[p,t,s,h(f)]  (4-dim ex broadcast)
            for sl in range(D_PAD):
                nc.vector.tensor_tensor(
                    out=ap4(wv, [[D_PAD * DIM, TC], [1, DIM]], extra_off=sl * DIM),
                    in0=ap4(kv_g, [[ROWW, TC], [1, DIM]], extra_off=sl * 128 + DIM),
                    in1=ap4(ex, [[D_PAD * H, TC], [1, H], [0, DK]], extra_off=sl * H),
                    op=mybir.AluOpType.mult,
                )
            ad = smal.tile([128, TC, 68], f16, tag="ad")
            nc.vector.tensor_tensor(
                out=ap4(wv, [[D_PAD * DIM, TC], [DIM, 4], [1, DIM]]),
                in0=ap4(wv, [[D_PAD * DIM, TC], [DIM, 4], [1, DIM]]),
                in1=ap4(wv, [[D_PAD * DIM, TC], [DIM, 4], [1, DIM]],
                        extra_off=4 * DIM),
                op=mybir.AluOpType.add,
            )
            nc.vector.tensor_tensor(
                out=ap4(wv, [[D_PAD * DIM, TC], [DIM, 2], [1, DIM]]),
                in0=ap4(wv, [[D_PAD * DIM, TC], [DIM, 2], [1, DIM]]),
                in1=ap4(wv, [[D_PAD * DIM, TC], [DIM, 2], [1, DIM]],
                        extra_off=2 * DIM),
                op=mybir.AluOpType.add,
            )
            nc.vector.tensor_tensor(
                out=ap4(ad, [[68, TC], [1, DIM]]),
                in0=ap4(wv, [[D_PAD * DIM, TC], [1, DIM]]),
                in1=ap4(wv, [[D_PAD * DIM, TC], [1, DIM]], extra_off=DIM),
                op=mybir.AluOpType.add,
            )
            nc.vector.tensor_tensor(
                out=ap4(ex, [[D_PAD * H, TC], [H, 4], [1, H]]),
                in0=ap4(ex, [[D_PAD * H, TC], [H, 4], [1, H]]),
                in1=ap4(ex, [[D_PAD * H, TC], [H, 4], [1, H]], extra_off=4 * H),
                op=mybir.AluOpType.add,
            )
            nc.vector.tensor_tensor(
                out=ap4(ex, [[D_PAD * H, TC], [H, 2], [1, H]]),
                in0=ap4(ex, [[D_PAD * H, TC], [H, 2], [1, H]]),
                in1=ap4(ex, [[D_PAD * H, TC], [H, 2], [1, H]], extra_off=2 * H),
                op=mybir.AluOpType.add,
            )
            nc.vector.tensor_tensor(
                out=ap4(ad, [[68, TC], [1, H]], extra_off=DIM),
                in0=ap4(ex, [[D_PAD * H, TC], [1, H]]),
                in1=ap4(ex, [[D_PAD * H, TC], [1, H]], extra_off=H),
                op=mybir.AluOpType.add,
            )
            oh_all = work.tile([128, TC, 128], f16, tag="oh")
            nc.vector.tensor_tensor(
                out=oh_all[:, :, :],
                in0=ap4(iota_f, [[0, TC], [1, 128]]),
                in1=ap4(d["crw"], [[1, TC], [0, 128]]),
                op=mybir.AluOpType.is_equal,
            )
            csb_all = outp.tile([128, TC, 68], f16, tag="csb")
            for t in range(TC):
                cps = ps.tile([128, 68], f32, space="PSUM", tag="x")
                nc.tensor.matmul(out=cps[:], lhsT=oh_all[:, t, :],
                                 rhs=ad[:, t, :], start=True, stop=True)
                nc.scalar.activation(
                    out=csb_all[:, t, :], in_=cps[:],
                    func=mybir.ActivationFunctionType.Copy,
                )
            nc.sync.dma_start(
                bass.AP(comb.ap()[:, :].tensor, st * TC * 128 * 68,
                        [[68, 128], [128 * 68, TC], [1, 68]]),
                csb_all[:, :, :],
            )
            if st < ST_A:
                nc.sync.dma_start(
                    bass.AP(comb_a.ap()[:, :].tensor, st * TC * 128 * 68,
                            [[68, 128], [128 * 68, TC], [1, 68]]),
                    csb_all[:, :, :],
                )

        nptr = consts.tile([128, NF], i32)
        nc.sync.dma_start(nptr[:], node_ptr[:, :])

        def f_chunk(fc, w, src_t):
            cgc = smal.tile([128, FCH, 68], f16, tag="cgc",
                            name=f"cgc{fc}")
            for f in range(w):
                nc.gpsimd.indirect_dma_start(
                    out=cgc[:, f, :], out_offset=None, in_=src_t.ap()[:, :],
                    in_offset=bass.IndirectOffsetOnAxis(
                        ap=nptr[:, fc + f : fc + f + 1], axis=0),
                )
            dn = smal.tile([128, FCH, H], f32, tag="dn", name=f"dn{fc}")
            nc.vector.tensor_scalar(
                out=dn[:, 0:w, :],
                in0=ap4(cgc, [[68, w], [1, H]], extra_off=DIM),
                scalar1=1e-30, scalar2=None,
                op0=mybir.AluOpType.max,
            )
            rd = smal.tile([128, FCH, H], f32, tag="rd", name=f"rd{fc}")
            nc.vector.reciprocal(rd[:, 0:w, :], dn[:, 0:w, :])
            nrm = outp.tile([128, FCH, DIM], f32, tag="nrm", name=f"nrm{fc}")
            nc.vector.tensor_tensor(
                out=nrm[:, 0:w, :],
                in0=ap4(cgc, [[68, w], [1, DIM]]),
                in1=ap4(rd, [[H, w], [1, H], [0, DK]]),
                op=mybir.AluOpType.mult,
            )
            osb = outp.tile([128, FCH, DIM], f32, tag="osb", name=f"osb{fc}")
            for f in range(w):
                tps = ps.tile([DIM, 128], f32, space="PSUM", tag="x",
                              name=f"tps{fc}_{f}")
                nc.tensor.transpose(out=tps[:], in_=nrm[:, f, :],
                                    identity=ident[:])
                nrmT = outp.tile([DIM, 128], f32, tag="nrmT",
                                 name=f"nrmT{fc}_{f}")
                nc.scalar.activation(
                    out=nrmT[:], in_=tps[:],
                    func=mybir.ActivationFunctionType.Copy,
                )
                ops_ = ps.tile([128, DIM], f32, space="PSUM", tag="mm",
                               name=f"ops{fc}_{f}")
                nc.tensor.matmul(out=ops_[:], lhsT=nrmT[:], rhs=wo_sb[:],
                                 start=True, stop=True)
                nc.vector.tensor_tensor(
                    out=osb[:, f, :], in0=ops_[:], in1=bo_sb[:],
                    op=mybir.AluOpType.add,
                )
            nc.sync.dma_start(
                bass.AP(out.tensor, fc * 128 * DIM,
                        [[DIM, 128], [128 * DIM, w], [1, DIM]]),
                osb[:, 0:w, :],
            )

        # software-pipelined main loop (2-supertile skew)
        WARM = min(2, ST)
        for st in range(WARM):
            stage_P(st)
            stage_A(st)
            stage_B(st)
        a_chunks = list(range(0, F_A, FCH))
        for st in range(ST):
            stage_C(st)
            if st >= ST_A and st % 2 == 1 and a_chunks:
                fc = a_chunks.pop(0)
                f_chunk(fc, min(FCH, NF - fc), comb_a)
            nx = st + WARM
            if nx < ST:
                stage_P(nx)
                stage_A(nx)
                stage_B(nx)
        for fc in a_chunks:
            f_chunk(fc, min(FCH, NF - fc), comb_a)

        # ---- Phase F chunks come from f_chunk(); A-chunks (early node
        # tiles, rows all inside comb_a) were emitted inside the main loop ----
        for fc in range(F_A, NF, FCH):
            f_chunk(fc, min(FCH, NF - fc), comb)

    nc.compile()
    return nc


def kernel(**inputs):
    from concourse.bass_utils import run_bass_kernel_spmd

    f16 = np.float16

    query = np.asarray(inputs["query"], np.float32)
    key = np.asarray(inputs["key"], np.float32)
    value = np.asarray(inputs["value"], np.float32)
    src = np.asarray(inputs["src"])
    dst = np.asarray(inputs["dst"])
    Wq = np.asarray(inputs["Wq"], np.float32)
    bq = np.asarray(inputs["bq"], np.float32)
    Wk = np.asarray(inputs["Wk"], np.float32)
    bk = np.asarray(inputs["bk"], np.float32)  # noqa: F841  (cancels in softmax)
    Wv = np.asarray(inputs["Wv"], np.float32)
    bv = np.asarray(inputs["bv"], np.float32)
    Wo = np.asarray(inputs["Wo"], np.float32)
    bo = np.asarray(inputs["bo"], np.float32)

    cores, NT, NODES_PAD, NF = _host_prep(src, dst, key, value, query)
    ST_A = (NT // TC) // 2
    F_A = 0
    if ST_A >= 1:
        lim = ST_A * TC  # all rows of nodes [0, F_A*128) must be in tiles < lim
        F_A = NF
        for c in cores:
            tf = c["tile_of"]
            nn = c["nn"]
            ok = np.nonzero(tf >= lim)[0]
            first_bad = int(ok[0]) if len(ok) else nn
            F_A = min(F_A, first_bad // 128)
        F_A = max(0, (F_A // FCH) * FCH)
    nc = _build_program(NT, NODES_PAD, NF, ST_A, F_A)

    wkv = np.zeros((128, 128), f16)
    wkv[0:DIM, 0:DIM] = Wk.T.astype(f16)
    wkv[DIM:128, DIM:128] = Wv.T.astype(f16)
    # bias folding: bk cancels in segment softmax; bv contributes Wo@bv to out
    boP = bo + Wo @ bv

    wq_aug = np.concatenate([Wq.T, bq[None, :]], axis=0).astype(f16)  # [65, 64]

    in_maps = []
    for c in cores:
        in_maps.append(
            dict(
                kvrawT=c["kvrawT"], qrawT=c["qrawT"], wkv=wkv,
                wqT=wq_aug.copy(),
                woT=Wo.T.copy(),
                bo=np.broadcast_to(boP, (128, DIM)).astype(np.float32).copy(),
                mask=c["mask"],
                crow=c["crow"], node_ptr=c["node_ptr"],
            )
        )

    trace = bool(int(os.environ.get("KERNEL_TRACE", "0")))
    res = run_bass_kernel_spmd(
        nc, in_maps, core_ids=list(range(NCORES)), trace=trace,
        tmpdir=os.environ.get("KERNEL_TRACE_DIR") or None,
    )
    kernel.last_results = res

    out = np.empty((N, DIM), np.float32)
    for c, r in zip(cores, res.results):
        out[c["n0"] : c["n1"]] = r["out"][: c["nn"]]
    # deg-0 nodes have sum(alpha) = 0, so the folded Wo@bv term must be removed
    deg = np.bincount(np.asarray(dst).astype(np.int64), minlength=N)
    z = deg == 0
    if z.any():
        out[z] -= Wo @ bv
    return out
